# revision 1
# baseline (speedup 1.0000x reference)
"""CNOT permutation kernel for Trainium2 (Bass), 8-core data parallel.

Problem (hardcoded from spec): state (16, 2**24) f32, control=3, target=10,
num_qubits=24.  With c2 = 24-3-1 = 20 and t2 = 24-10-1 = 13:

    out[b, j] = state[b, j ^ (1<<13)]  if (j >> 20) & 1 else state[b, j]

Pure data movement.  Viewing the per-core shard flat (row stride 2**24 is a
multiple of the 2**21 control-bit period, so both rows fuse) as
[blk:16, ctrl:2, c:64, d:2, e:8192]:

    out[blk, 0, c, d, e] = in[blk, 0, c, d, e]      (identity half)
    out[blk, 1, c, d, e] = in[blk, 1, c, 1-d, e]    (swap 8192-elem chunk pairs)

Sharding: batch axis, 2 rows per core (pure data parallel).

Two device kernels, both HBM->SBUF->HBM bounces (direct DRAM->DRAM DMA only
sustains ~5.8 GB/s per SDMA engine because the read and write serialize
inside the engine; split load/store descriptors stream at ~21-25 GB/s per
engine).  Loads issue on the Sync HWDGE ring, stores on the Scalar ring; the
16 SDMA engines round-robin between rings at packet granularity so both HBM
directions stay busy.  4 MiB slabs ([128, 8192] f32 tiles), NBUF-deep manual
double buffering with two semaphores (RAW: store waits its slab's load; WAR:
load waits the store that previously used its SBUF slot).  The chunk-pair
swap is expressed in the store APs: even partitions (d=0 data) store to d=1
positions and vice versa.

1. In-place (default): a single DRAM tensor, pre-initialized with the input
   shard by donating it as the PJRT output buffer (the same donation
   mechanism run_bass_via_pjrt uses for its zero-filled outputs).  Only the
   control-bit=1 half is touched: 16 swap slabs -> 128 MiB of HBM traffic
   per core.  The identity half is never moved at all.
2. Full-copy (fallback): separate in/out tensors, all 32 slabs move through
   SBUF -> 256 MiB of HBM traffic per core.

kernel() runs the in-place path and sample-checks the permutation against
the host input; if the donation aliasing ever fails to hold (output buffer
not seeded with the input), it falls back to the full-copy path.
"""

import numpy as np

import concourse.bass as bass
import concourse.mybir as mybir
from concourse.bass_utils import run_bass_kernel_spmd

NUM_QUBITS = 24
DIM = 1 << NUM_QUBITS
BATCH = 16
N_CORES = 8
ROWS = BATCH // N_CORES  # 2 rows per core
C2 = NUM_QUBITS - 3 - 1  # 20
T2 = NUM_QUBITS - 10 - 1  # 13
CBIT = 1 << C2  # 1048576 elements (4 MiB)
TBIT = 1 << T2  # 8192 elements (32 KiB)
BLK = 2 * CBIT  # control-bit period
NBLK = ROWS * DIM // BLK  # 16 blocks in the fused per-core space

P = 128
FREE = CBIT // P  # 8192: slab is [128, 8192] f32 = 4 MiB
NBUF = 6

_cache = {}


def _emit_bounce(nc, src, dst, slabs):
    """Loads on sync / stores on scalar, NBUF-deep pipeline over slabs."""
    n = len(slabs)
    with (
        nc.sbuf_tensor("tiles", [P, NBUF * FREE], mybir.dt.float32) as tiles,
        nc.semaphore("load_sem") as load_sem,
        nc.semaphore("store_sem") as store_sem,
        nc.Block() as block,
    ):

        def tile_view(i):
            return tiles[:, (i % NBUF) * FREE : (i % NBUF + 1) * FREE]

        @block.sync
        def _(sync):
            for i, (base, _swap) in enumerate(slabs):
                if i >= NBUF:
                    sync.wait_ge(store_sem, 32 * (i - NBUF + 1))
                sync.dma_start(
                    out=tile_view(i),
                    in_=bass.AP(src, base, [[1, CBIT]]),
                ).then_inc(load_sem, 16)

        @block.scalar
        def _(scalar):
            for i, (base, swap) in enumerate(slabs):
                scalar.wait_ge(load_sem, 16 * (i + 1))
                t = tile_view(i)
                if swap:
                    # partition p holds chunk (c, d) with p = 2c + d
                    scalar.dma_start(
                        out=bass.AP(dst, base + TBIT, [[2 * TBIT, P // 2], [1, TBIT]]),
                        in_=t[0::2, :],  # d=0 data -> d=1 positions
                    ).then_inc(store_sem, 16)
                    scalar.dma_start(
                        out=bass.AP(dst, base, [[2 * TBIT, P // 2], [1, TBIT]]),
                        in_=t[1::2, :],  # d=1 data -> d=0 positions
                    ).then_inc(store_sem, 16)
                else:
                    scalar.dma_start(
                        out=bass.AP(dst, base, [[1, CBIT // 2]]),
                        in_=t[0 : P // 2, :],
                    ).then_inc(store_sem, 16)
                    scalar.dma_start(
                        out=bass.AP(dst, base + CBIT // 2, [[1, CBIT // 2]]),
                        in_=t[P // 2 :, :],
                    ).then_inc(store_sem, 16)
            scalar.wait_ge(store_sem, 32 * n)


def _build_nc(inplace):
    nc = bass.Bass(target_bir_lowering=False)
    out = nc.dram_tensor("out", (ROWS, DIM), mybir.dt.float32, kind="ExternalOutput")
    if inplace:
        slabs = [(b * BLK + CBIT, True) for b in range(NBLK)]
        _emit_bounce(nc, out, out, slabs)
    else:
        st = nc.dram_tensor(
            "state", (ROWS, DIM), mybir.dt.float32, kind="ExternalInput"
        )
        slabs = []
        for b in range(NBLK):
            slabs.append((b * BLK, False))
            slabs.append((b * BLK + CBIT, True))
        _emit_bounce(nc, st, out, slabs)
    if not nc.is_finalized():
        nc.finalize()
    return nc


def _get_nc(inplace):
    key = ("ip" if inplace else "fc",)
    if key not in _cache:
        _cache[key] = _build_nc(inplace)
    return _cache[key]


def _run_donated(nc, state):
    """Run `nc` via PJRT shard_map over 8 cores, donating the input state as
    the initial content of the (aliased) output buffer — the same donation
    mechanism run_bass_via_pjrt uses for its zero-filled output buffers."""
    import jax
    from jax.experimental.shard_map import shard_map
    from jax.sharding import Mesh, PartitionSpec

    from concourse.bass2jax import (
        _bass_exec_p,
        install_neuronx_cc_hook,
        partition_id_tensor,
    )

    install_neuronx_cc_hook()

    out_names, out_avals = [], []
    for alloc in nc.m.functions[0].allocations:
        if (
            isinstance(alloc, mybir.MemoryLocationSet)
            and alloc.kind == "ExternalOutput"
        ):
            out_names.append(alloc.memorylocations[0].name)
            out_avals.append(
                jax.core.ShapedArray(
                    tuple(alloc.tensor_shape), mybir.dt.np(alloc.dtype)
                )
            )
    partition_name = nc.partition_id_tensor.name if nc.partition_id_tensor else None
    in_names = list(out_names)
    if partition_name is not None:
        in_names.append(partition_name)

    if "donated_fn" not in _cache:

        def _body(buf):
            operands = [buf]
            if partition_name is not None:
                operands.append(partition_id_tensor())
            outs = _bass_exec_p.bind(
                *operands,
                out_avals=tuple(out_avals),
                in_names=tuple(in_names),
                out_names=tuple(out_names),
                lowering_input_output_aliases=(),
                sim_require_finite=True,
                sim_require_nnan=True,
                nc=nc,
            )
            return outs[0]

        devices = jax.devices()[:N_CORES]
        mesh = Mesh(np.asarray(devices), ("core",))
        _cache["donated_fn"] = jax.jit(
            shard_map(
                _body,
                mesh=mesh,
                in_specs=(PartitionSpec("core"),),
                out_specs=PartitionSpec("core"),
                check_rep=False,
            ),
            donate_argnums=(0,),
            keep_unused=True,
        )

    out = _cache["donated_fn"](state)
    return np.asarray(out)


def _sample_ok(state, out, rng, k=2048):
    """Spot-check out[b, j] == state[b, j ^ (1<<13) if bit20(j) else j]."""
    b = rng.integers(0, BATCH, size=k)
    j = rng.integers(0, DIM, size=k)
    src = np.where((j >> C2) & 1 == 1, j ^ TBIT, j)
    return np.array_equal(out[b, j], state[b, src])


def kernel(state, control=3, target=10, num_qubits=24, **_):
    state = np.ascontiguousarray(np.asarray(state, dtype=np.float32))
    assert state.shape == (BATCH, DIM), state.shape
    assert int(control) == 3 and int(target) == 10 and int(num_qubits) == 24

    rng = np.random.default_rng(0)
    try:
        out = _run_donated(_get_nc(inplace=True), state)
        if _sample_ok(state, out, rng):
            return out
    except Exception:
        pass

    # Fallback: full-copy kernel through run_bass_kernel_spmd.
    nc = _get_nc(inplace=False)
    in_maps = [{"state": state[c * ROWS : (c + 1) * ROWS]} for c in range(N_CORES)]
    res = run_bass_kernel_spmd(nc, in_maps, core_ids=list(range(N_CORES)))
    return np.concatenate([r["out"] for r in res.results], axis=0)



# revision 2
# speedup vs baseline: 1.8867x; 1.8867x over previous
"""CNOT permutation kernel for Trainium2 (Bass), 8-core data parallel. v2.

Problem (hardcoded from spec): state (16, 2**24) f32, control=3, target=10,
num_qubits=24.  With c2 = 24-3-1 = 20 and t2 = 24-10-1 = 13:

    out[b, j] = state[b, j ^ (1<<13)]  if (j >> 20) & 1 else state[b, j]

Pure data movement.  Viewing the per-core shard flat (row stride 2**24 is a
multiple of the 2**21 control-bit period, so both rows fuse) as
[blk:16, ctrl:2, c:64, d:2, e:8192]:

    out[blk, 0, c, d, e] = in[blk, 0, c, d, e]      (identity half)
    out[blk, 1, c, d, e] = in[blk, 1, c, 1-d, e]    (swap 8192-elem chunk pairs)

Sharding: batch axis, 2 rows per core (pure data parallel).

In-place path (primary): a single DRAM tensor pre-seeded with the input
shard by donating it as the PJRT output buffer.  Only the control-bit=1
half moves: 16 swap slabs of 4 MiB -> 64 MiB read + 64 MiB write per core.
The identity half never moves.  HBM floor with both NCs of a stack running
concurrently: 2 x 134.2 MB / 716 GB/s = 375 us.

v2 pipeline: the two HWDGE rings (sync, scalar) each own half the slabs
end-to-end (load + swapped store), software-pipelined NBUF/2 deep per ring
with private semaphores.  Both descriptor generators fill the 16 shared
SDMA engines concurrently, halving the startup ramp vs v1's
loads-on-sync/stores-on-scalar split.  The chunk-pair swap is expressed in
the store APs: even partitions (d=0 data) store to d=1 positions and vice
versa (slab tile [128, 8192]: partition p holds exactly chunk (c, d) with
p = 2c + d, one 32 KiB chunk per partition).

Full-copy fallback: separate in/out tensors, all 32 slabs bounce through
SBUF (identity slabs stored straight, swap slabs stored swapped) -> 256 MiB
of SDMA payload per core, HBM-stack floor 750 us.  Runs only if the
donation aliasing fails to hold (checked by sampling) or the donated run
raises.
"""

import numpy as np

import concourse.bass as bass
import concourse.mybir as mybir
from concourse.bass_utils import run_bass_kernel_spmd

NUM_QUBITS = 24
DIM = 1 << NUM_QUBITS
BATCH = 16
N_CORES = 8
ROWS = BATCH // N_CORES  # 2 rows per core
C2 = NUM_QUBITS - 3 - 1  # 20
T2 = NUM_QUBITS - 10 - 1  # 13
CBIT = 1 << C2  # 1048576 elements (4 MiB)
TBIT = 1 << T2  # 8192 elements (32 KiB)
BLK = 2 * CBIT  # control-bit period
NBLK = ROWS * DIM // BLK  # 16 blocks in the fused per-core space

P = 128
FREE = CBIT // P  # 8192: slab is [128, 8192] f32 = 4 MiB
NBUF = 6  # total slab buffers (split across the two rings)

_cache = {}


def _emit_ring(engine_stream, src, dst, slabs, tiles, load_sem, store_sem, depth):
    """One HWDGE ring owns `slabs` end-to-end: software pipeline `depth` deep.

    Stream order: prime `depth` loads, then for each slab wait its load and
    issue the swapped/straight store pair, then issue the next load (which
    reuses the buffer of the slab `depth` back -- guarded by waiting that
    slab's stores).
    """
    n = len(slabs)

    def tile_view(i):
        return tiles[:, (i % depth) * FREE : (i % depth + 1) * FREE]

    def load(i):
        base, _swap = slabs[i]
        engine_stream.dma_start(
            out=tile_view(i),
            in_=bass.AP(src, base, [[1, CBIT]]),
        ).then_inc(load_sem, 16)

    for i in range(min(depth, n)):
        load(i)
    for i in range(n):
        base, swap = slabs[i]
        engine_stream.wait_ge(load_sem, 16 * (i + 1))
        t = tile_view(i)
        if swap:
            # partition p holds chunk (c, d) with p = 2c + d
            engine_stream.dma_start(
                out=bass.AP(dst, base + TBIT, [[2 * TBIT, P // 2], [1, TBIT]]),
                in_=t[0::2, :],  # d=0 data -> d=1 positions
            ).then_inc(store_sem, 16)
            engine_stream.dma_start(
                out=bass.AP(dst, base, [[2 * TBIT, P // 2], [1, TBIT]]),
                in_=t[1::2, :],  # d=1 data -> d=0 positions
            ).then_inc(store_sem, 16)
        else:
            engine_stream.dma_start(
                out=bass.AP(dst, base, [[1, CBIT // 2]]),
                in_=t[0 : P // 2, :],
            ).then_inc(store_sem, 16)
            engine_stream.dma_start(
                out=bass.AP(dst, base + CBIT // 2, [[1, CBIT // 2]]),
                in_=t[P // 2 :, :],
            ).then_inc(store_sem, 16)
        if i + depth < n:
            # buffer reuse guard: slab i+depth overwrites slab i's buffer...
            # wait for slab (i+depth-depth)=i's stores? No: the next load
            # reuses the buffer of slab i+depth-depth = i, whose stores were
            # just issued on THIS stream -- HWDGE FIFO per ring means the
            # store descriptors are generated before the load's, but the SDMA
            # engines may reorder across queues. Wait for the stores of the
            # slab whose buffer is being reused before issuing the load.
            engine_stream.wait_ge(store_sem, 32 * (i + 1))
            load(i + depth)
    engine_stream.wait_ge(store_sem, 32 * n)


def _emit_bounce2(nc, src, dst, slabs):
    """Dual-ring pipeline: sync ring owns even-half slabs, scalar ring the
    rest.  Each ring loads and stores its own slabs (private sems/buffers);
    the two descriptor generators fill the 16 shared SDMA engines
    concurrently."""
    half = (len(slabs) + 1) // 2
    slabs_a, slabs_b = slabs[:half], slabs[half:]
    depth = NBUF // 2
    with (
        nc.sbuf_tensor("tiles_a", [P, depth * FREE], mybir.dt.float32) as tiles_a,
        nc.sbuf_tensor("tiles_b", [P, depth * FREE], mybir.dt.float32) as tiles_b,
        nc.semaphore("load_sem_a") as load_sem_a,
        nc.semaphore("store_sem_a") as store_sem_a,
        nc.semaphore("load_sem_b") as load_sem_b,
        nc.semaphore("store_sem_b") as store_sem_b,
        nc.Block() as block,
    ):

        @block.sync
        def _(sync):
            _emit_ring(sync, src, dst, slabs_a, tiles_a, load_sem_a, store_sem_a, depth)

        @block.scalar
        def _(scalar):
            _emit_ring(
                scalar, src, dst, slabs_b, tiles_b, load_sem_b, store_sem_b, depth
            )


def _build_nc(inplace):
    nc = bass.Bass(target_bir_lowering=False)
    out = nc.dram_tensor("out", (ROWS, DIM), mybir.dt.float32, kind="ExternalOutput")
    if inplace:
        slabs = [(b * BLK + CBIT, True) for b in range(NBLK)]
        _emit_bounce2(nc, out, out, slabs)
    else:
        st = nc.dram_tensor(
            "state", (ROWS, DIM), mybir.dt.float32, kind="ExternalInput"
        )
        slabs = []
        for b in range(NBLK):
            slabs.append((b * BLK, False))
            slabs.append((b * BLK + CBIT, True))
        # interleave so both rings get an even identity/swap mix
        _emit_bounce2(nc, st, out, slabs)
    if not nc.is_finalized():
        nc.finalize()
    return nc


def _get_nc(inplace):
    key = ("ip" if inplace else "fc",)
    if key not in _cache:
        _cache[key] = _build_nc(inplace)
    return _cache[key]


def _run_donated(nc, state):
    """Run `nc` via PJRT shard_map over 8 cores, donating the input state as
    the initial content of the (aliased) output buffer -- the same donation
    mechanism run_bass_via_pjrt uses for its zero-filled outputs."""
    import jax

    try:
        from jax.experimental.shard_map import shard_map
    except ImportError:  # moved in newer jax
        from jax import shard_map
    from jax.sharding import Mesh, PartitionSpec

    from concourse.bass2jax import (
        _bass_exec_p,
        install_neuronx_cc_hook,
        partition_id_tensor,
    )

    install_neuronx_cc_hook()

    try:
        shaped_array = jax.core.ShapedArray
    except AttributeError:  # moved in newer jax
        from jax._src.core import ShapedArray as shaped_array

    out_names, out_avals = [], []
    for alloc in nc.m.functions[0].allocations:
        if (
            isinstance(alloc, mybir.MemoryLocationSet)
            and alloc.kind == "ExternalOutput"
        ):
            out_names.append(alloc.memorylocations[0].name)
            out_avals.append(
                shaped_array(tuple(alloc.tensor_shape), mybir.dt.np(alloc.dtype))
            )
    partition_name = nc.partition_id_tensor.name if nc.partition_id_tensor else None
    in_names = list(out_names)
    if partition_name is not None:
        in_names.append(partition_name)

    if "donated_fn" not in _cache:

        def _body(buf):
            operands = [buf]
            if partition_name is not None:
                operands.append(partition_id_tensor())
            outs = _bass_exec_p.bind(
                *operands,
                out_avals=tuple(out_avals),
                in_names=tuple(in_names),
                out_names=tuple(out_names),
                lowering_input_output_aliases=(),
                sim_require_finite=True,
                sim_require_nnan=True,
                nc=nc,
            )
            return outs[0]

        devices = jax.devices()[:N_CORES]
        mesh = Mesh(np.asarray(devices), ("core",))
        _cache["donated_fn"] = jax.jit(
            shard_map(
                _body,
                mesh=mesh,
                in_specs=(PartitionSpec("core"),),
                out_specs=PartitionSpec("core"),
                check_rep=False,
            ),
            donate_argnums=(0,),
            keep_unused=True,
        )

    out = _cache["donated_fn"](state)
    return np.asarray(out)


def _sample_ok(state, out, rng, k=2048):
    """Spot-check out[b, j] == state[b, j ^ (1<<13) if bit20(j) else j]."""
    b = rng.integers(0, BATCH, size=k)
    j = rng.integers(0, DIM, size=k)
    src = np.where((j >> C2) & 1 == 1, j ^ TBIT, j)
    return np.array_equal(out[b, j], state[b, src])


def kernel(state, control=3, target=10, num_qubits=24, **_):
    state = np.ascontiguousarray(np.asarray(state, dtype=np.float32))
    assert state.shape == (BATCH, DIM), state.shape
    assert int(control) == 3 and int(target) == 10 and int(num_qubits) == 24

    rng = np.random.default_rng(0)
    # two attempts: donation failures can be transient (buffer held elsewhere)
    for _attempt in range(2):
        try:
            out = _run_donated(_get_nc(inplace=True), state)
            if _sample_ok(state, out, rng):
                return out
        except Exception:
            pass

    # Fallback: full-copy kernel through run_bass_kernel_spmd.
    nc = _get_nc(inplace=False)
    in_maps = [{"state": state[c * ROWS : (c + 1) * ROWS]} for c in range(N_CORES)]
    res = run_bass_kernel_spmd(nc, in_maps, core_ids=list(range(N_CORES)))
    return np.concatenate([r["out"] for r in res.results], axis=0)


# revision 5
# speedup vs baseline: 2.2954x; 1.2166x over previous
"""CNOT permutation kernel for Trainium2 (Bass), 8-core data parallel.

Problem (hardcoded from spec): state (16, 2**24) f32, control=3, target=10,
num_qubits=24.  With c2 = 24-3-1 = 20 and t2 = 24-10-1 = 13:

    out[b, j] = state[b, j ^ (1<<13)]  if (j >> 20) & 1 else state[b, j]

Pure data movement.  Viewing the per-core shard flat (row stride 2**24 is a
multiple of the 2**21 control-bit period, so both rows fuse) as
[blk:16, ctrl:2, c:64, d:2, e:8192]:

    out[blk, 0, c, d, e] = in[blk, 0, c, d, e]      (identity half)
    out[blk, 1, c, d, e] = in[blk, 1, c, 1-d, e]    (swap 8192-elem chunk pairs)

Sharding: batch axis, 2 rows per core (pure data parallel).

Two device kernels, both HBM->SBUF->HBM bounces (direct DRAM->DRAM DMA only
sustains ~5.8 GB/s per SDMA engine because the read and write serialize
inside the engine; split load/store descriptors stream at ~21-25 GB/s per
engine).  Loads issue on the Sync HWDGE ring, stores on the Scalar ring; the
16 SDMA engines round-robin between rings at packet granularity so both HBM
directions stay busy.  4 MiB slabs ([128, 8192] f32 tiles), NBUF-deep manual
double buffering with two semaphores (RAW: store waits its slab's load; WAR:
load waits the store that previously used its SBUF slot).  The chunk-pair
swap is expressed in the store APs: even partitions (d=0 data) store to d=1
positions and vice versa.

1. In-place (default): a single DRAM tensor, pre-initialized with the input
   shard by donating it as the PJRT output buffer (the same donation
   mechanism run_bass_via_pjrt uses for its zero-filled outputs).  Only the
   control-bit=1 half is touched: 16 swap slabs -> 128 MiB of HBM traffic
   per core.  The identity half is never moved at all.
2. Full-copy (fallback): separate in/out tensors, all 32 slabs move through
   SBUF -> 256 MiB of HBM traffic per core.

kernel() runs the in-place path and sample-checks the permutation against
the host input; if the donation aliasing ever fails to hold (output buffer
not seeded with the input), it falls back to the full-copy path.
"""

import numpy as np

import concourse.bass as bass
import concourse.mybir as mybir
from concourse.bass_utils import run_bass_kernel_spmd

NUM_QUBITS = 24
DIM = 1 << NUM_QUBITS
BATCH = 16
N_CORES = 8
ROWS = BATCH // N_CORES  # 2 rows per core
C2 = NUM_QUBITS - 3 - 1  # 20
T2 = NUM_QUBITS - 10 - 1  # 13
CBIT = 1 << C2  # 1048576 elements (4 MiB)
TBIT = 1 << T2  # 8192 elements (32 KiB)
BLK = 2 * CBIT  # control-bit period
NBLK = ROWS * DIM // BLK  # 16 blocks in the fused per-core space

P = 128
FREE = CBIT // P  # 8192: slab is [128, 8192] f32 = 4 MiB
NBUF = 6

_cache = {}


def _emit_bounce(nc, src, dst, slabs):
    """Loads on sync / stores on scalar, NBUF-deep pipeline over slabs."""
    n = len(slabs)
    with (
        nc.sbuf_tensor("tiles", [P, NBUF * FREE], mybir.dt.float32) as tiles,
        nc.semaphore("load_sem") as load_sem,
        nc.semaphore("store_sem") as store_sem,
        nc.Block() as block,
    ):

        def tile_view(i):
            return tiles[:, (i % NBUF) * FREE : (i % NBUF + 1) * FREE]

        @block.sync
        def _(sync):
            for i, (base, _swap) in enumerate(slabs):
                if i >= NBUF:
                    sync.wait_ge(store_sem, 32 * (i - NBUF + 1))
                sync.dma_start(
                    out=tile_view(i),
                    in_=bass.AP(src, base, [[1, CBIT]]),
                ).then_inc(load_sem, 16)

        @block.scalar
        def _(scalar):
            for i, (base, swap) in enumerate(slabs):
                scalar.wait_ge(load_sem, 16 * (i + 1))
                t = tile_view(i)
                if swap:
                    # partition p holds chunk (c, d) with p = 2c + d
                    scalar.dma_start(
                        out=bass.AP(dst, base + TBIT, [[2 * TBIT, P // 2], [1, TBIT]]),
                        in_=t[0::2, :],  # d=0 data -> d=1 positions
                    ).then_inc(store_sem, 16)
                    scalar.dma_start(
                        out=bass.AP(dst, base, [[2 * TBIT, P // 2], [1, TBIT]]),
                        in_=t[1::2, :],  # d=1 data -> d=0 positions
                    ).then_inc(store_sem, 16)
                else:
                    scalar.dma_start(
                        out=bass.AP(dst, base, [[1, CBIT // 2]]),
                        in_=t[0 : P // 2, :],
                    ).then_inc(store_sem, 16)
                    scalar.dma_start(
                        out=bass.AP(dst, base + CBIT // 2, [[1, CBIT // 2]]),
                        in_=t[P // 2 :, :],
                    ).then_inc(store_sem, 16)
            scalar.wait_ge(store_sem, 32 * n)


def _build_nc(inplace):
    nc = bass.Bass(target_bir_lowering=False)
    out = nc.dram_tensor("out", (ROWS, DIM), mybir.dt.float32, kind="ExternalOutput")
    if inplace:
        slabs = [(b * BLK + CBIT, True) for b in range(NBLK)]
        _emit_bounce(nc, out, out, slabs)
    else:
        st = nc.dram_tensor(
            "state", (ROWS, DIM), mybir.dt.float32, kind="ExternalInput"
        )
        slabs = []
        for b in range(NBLK):
            slabs.append((b * BLK, False))
            slabs.append((b * BLK + CBIT, True))
        _emit_bounce(nc, st, out, slabs)
    if not nc.is_finalized():
        nc.finalize()
    return nc


def _get_nc(inplace):
    key = ("ip" if inplace else "fc",)
    if key not in _cache:
        _cache[key] = _build_nc(inplace)
    return _cache[key]


def _run_donated(nc, state):
    """Run `nc` via PJRT shard_map over 8 cores, donating the input state as
    the initial content of the (aliased) output buffer — the same donation
    mechanism run_bass_via_pjrt uses for its zero-filled output buffers."""
    import jax

    try:
        from jax.experimental.shard_map import shard_map
    except ImportError:  # moved in newer jax
        from jax import shard_map
    from jax.sharding import Mesh, PartitionSpec

    from concourse.bass2jax import (
        _bass_exec_p,
        install_neuronx_cc_hook,
        partition_id_tensor,
    )

    install_neuronx_cc_hook()

    try:
        shaped_array = jax.core.ShapedArray
    except AttributeError:  # moved in newer jax
        from jax._src.core import ShapedArray as shaped_array

    out_names, out_avals = [], []
    for alloc in nc.m.functions[0].allocations:
        if (
            isinstance(alloc, mybir.MemoryLocationSet)
            and alloc.kind == "ExternalOutput"
        ):
            out_names.append(alloc.memorylocations[0].name)
            out_avals.append(
                shaped_array(tuple(alloc.tensor_shape), mybir.dt.np(alloc.dtype))
            )
    partition_name = nc.partition_id_tensor.name if nc.partition_id_tensor else None
    in_names = list(out_names)
    if partition_name is not None:
        in_names.append(partition_name)

    if "donated_fn" not in _cache:

        def _body(buf):
            operands = [buf]
            if partition_name is not None:
                operands.append(partition_id_tensor())
            outs = _bass_exec_p.bind(
                *operands,
                out_avals=tuple(out_avals),
                in_names=tuple(in_names),
                out_names=tuple(out_names),
                lowering_input_output_aliases=(),
                sim_require_finite=True,
                sim_require_nnan=True,
                nc=nc,
            )
            return outs[0]

        devices = jax.devices()[:N_CORES]
        mesh = Mesh(np.asarray(devices), ("core",))
        _cache["donated_fn"] = jax.jit(
            shard_map(
                _body,
                mesh=mesh,
                in_specs=(PartitionSpec("core"),),
                out_specs=PartitionSpec("core"),
                check_rep=False,
            ),
            donate_argnums=(0,),
            keep_unused=True,
        )

    out = _cache["donated_fn"](state)
    return np.asarray(out)


def _sample_ok(state, out, rng, k=2048):
    """Spot-check out[b, j] == state[b, j ^ (1<<13) if bit20(j) else j]."""
    b = rng.integers(0, BATCH, size=k)
    j = rng.integers(0, DIM, size=k)
    src = np.where((j >> C2) & 1 == 1, j ^ TBIT, j)
    return np.array_equal(out[b, j], state[b, src])


def kernel(state, control=3, target=10, num_qubits=24, **_):
    state = np.ascontiguousarray(np.asarray(state, dtype=np.float32))
    assert state.shape == (BATCH, DIM), state.shape
    assert int(control) == 3 and int(target) == 10 and int(num_qubits) == 24

    rng = np.random.default_rng(0)
    # two attempts: donation failures can be transient (buffer held elsewhere)
    for _attempt in range(2):
        try:
            out = _run_donated(_get_nc(inplace=True), state)
            if _sample_ok(state, out, rng):
                return out
        except Exception:
            pass

    # Fallback: full-copy kernel through run_bass_kernel_spmd.
    nc = _get_nc(inplace=False)
    in_maps = [{"state": state[c * ROWS : (c + 1) * ROWS]} for c in range(N_CORES)]
    res = run_bass_kernel_spmd(nc, in_maps, core_ids=list(range(N_CORES)))
    return np.concatenate([r["out"] for r in res.results], axis=0)

